# revision 9
# baseline (speedup 1.0000x reference)
"""Distributed cosine-attention kernel for TRN2 (8 NeuronCores).

Problem (nn_Attention): B=4, N=2048, D_MODEL=1024, HEADS=16, DIM_HEAD=64
  qkv = x @ w_qkv.T + b_qkv ; q,k l2-normalized over head dim;
  attn = softmax(clip-scale * qn @ kn^T); out = (attn @ v) @ w_out.T

Sharding (tensor-parallel heads x data-parallel batch):
  core c handles batch b=c//2 and global heads hg*8..hg*8+8 (hg=c%2).
  Each core computes a partial out^T [D_MODEL, N]; the host sums the two
  cores of each batch and transposes.

Per-core dataflow (layouts chosen so no on-device transposes are needed):
  - host passes x[b].T ("xT" [C,T]) and pre-transposed weight shards
  - QK proj: lhsT=wqkT tiles -> Q^T/K^T [d-on-partition, tok-free]; the two
    heads of a pair occupy partitions 0:64 / 64:128 of one tile
  - V proj: lhsT=xT tiles -> V [tok-on-partition, d-free]; bias via K=1 matmul
  - l2norm: Square (ACT), per-head sum over d via K=2 mask matmul,
    1/x (DVE approx) -> sqrt (ACT) -> broadcast via K=2 matmul -> multiply;
    per-head logit scale s folded into K^T
  - S^T tiles [keys, queries] via f32r row-packed matmuls (two K=64 heads
    run concurrently in row groups 0/64)
  - P^T = Exp(S^T - s) on ScalarE over multi-bank PSUM groups, bf16 out
  - O^T = P@V via bf16 col-packed matmuls (tile_position (0,0)/(0,64))
    accumulating over all key tiles in one PSUM bank; softmax denominator
    via ones-stationary col-packed matmuls producing broadcast rows
  - out^T partial = woutT tiles @ O (bf16)
"""
import sys
sys.path.insert(0, "/opt/trn_rl_repo")

from dataclasses import dataclass, field

import numpy as np

try:
    import ml_dtypes
    ml_bf16 = ml_dtypes.bfloat16
except ImportError:  # pragma: no cover
    ml_bf16 = np.float32

import concourse.bass as bass
import concourse.tile as tile
import concourse.mybir as mybir
from concourse import bacc
from concourse.bass_utils import run_bass_kernel_spmd

F32 = mybir.dt.float32
F32R = mybir.dt.float32r
BF16 = mybir.dt.bfloat16
AF = mybir.ActivationFunctionType

D_MODEL = 1024
HEADS = 16
DIM_HEAD = 64
INNER = HEADS * DIM_HEAD
B = 4
N = 2048
N_CORES = 8
LOG100 = float(np.log(100.0))


@dataclass
class Cfg:
    T: int = N              # tokens per core (one batch element)
    C: int = D_MODEL        # d_model
    NH: int = 8             # local heads per core
    DH: int = DIM_HEAD
    QB: int = 512           # query block (free dim of S^T matmuls)
    SG: int = 2             # k-tiles per exp group
    merge_pairs: tuple = (True, True, True, True)  # per-pair merged exp

    @property
    def PAIRS(self):
        return self.NH // 2

    @property
    def CT(self):
        return self.C // 128

    @property
    def KT(self):
        return self.T // 128

    @property
    def NQB(self):
        return self.T // self.QB

    @property
    def VW(self):
        return self.NH * self.DH  # local inner width


def build(cfg: Cfg):
    T, C, QB = cfg.T, cfg.C, cfg.QB
    PAIRS, CT, KT, NQB, VW = cfg.PAIRS, cfg.CT, cfg.KT, cfg.NQB, cfg.VW
    SG = cfg.SG

    nc = bacc.Bacc("TRN2", target_bir_lowering=False, debug=False,
                   enable_asserts=False)

    xT_d = nc.declare_dram_parameter("xT", [C, T], F32R, isOutput=False)
    wqkT_d = nc.declare_dram_parameter("wqkT", [C, 2 * PAIRS * 128], F32R,
                                       isOutput=False)
    bqk_d = nc.declare_dram_parameter("bqk", [2 * PAIRS, 128, 1], F32,
                                      isOutput=False)
    wvT_d = nc.declare_dram_parameter("wvT", [C, VW], F32R, isOutput=False)
    bv_d = nc.declare_dram_parameter("bv", [1, VW], F32R, isOutput=False)
    woT_d = nc.declare_dram_parameter("woT", [VW, C], BF16, isOutput=False)
    # per-head scale constants: [:, 0]=-s_h (exp bias), [:, 1]=s_h^2
    scl_d = nc.declare_dram_parameter("scl", [cfg.NH, 2, 1], F32,
                                      isOutput=False)
    sel2_d = nc.declare_dram_parameter("sel2", [2, 128], F32, isOutput=False)
    sel2T_d = nc.declare_dram_parameter("sel2T", [128, 2], F32,
                                        isOutput=False)
    out_d = nc.declare_dram_parameter("out", [C, T], F32, isOutput=True)

    with tile.TileContext(nc) as tc:
        with (
            tc.tile_pool(name="const", bufs=1) as const,
            tc.tile_pool(name="dram", bufs=1, space="DRAM") as dram,
            tc.tile_pool(name="xt", bufs=1) as xt_pool,
            tc.tile_pool(name="wcol", bufs=2) as wcol_pool,
            tc.tile_pool(name="qksb", bufs=2) as qk_sb,
            tc.tile_pool(name="norm", bufs=2) as norm_sb,
            tc.tile_pool(name="vtmp", bufs=3) as vtmp_pool,
            tc.tile_pool(name="att", bufs=2) as att_sb,
            tc.tile_pool(name="pt", bufs=3) as pt_pool,
            tc.tile_pool(name="ofin", bufs=1) as ofin_pool,
            tc.tile_pool(name="otout", bufs=3) as ot_pool,
            # PSUM budget (8 banks): mm 2 + sg 4 + pv 1 + lb 1
            tc.tile_pool(name="psmm", bufs=2, space="PSUM") as ps_mm,
            tc.tile_pool(name="pssg", bufs=1, space="PSUM") as ps_sg,
            tc.tile_pool(name="pspv", bufs=1, space="PSUM") as ps_pv,
        ):
            # ---- DRAM spill tensors ----
            qhat_sp = [dram.tile([128, T], F32R, tag=f"qsp{p}", name=f"qsp{p}")
                       for p in range(PAIRS)]
            khat_sp = [dram.tile([128, T], F32R, tag=f"ksp{p}", name=f"ksp{p}")
                       for p in range(PAIRS)]
            vhat_sp = dram.tile([KT, 128, VW], BF16, tag="vsp")

            # ---- constants ----
            scratch_f = const.tile([128, 128], F32, tag="scratch")
            nc.vector.memset(scratch_f, 1.0)
            ones_bf = const.tile([128, 64], BF16, tag="ones_bf")
            nc.vector.tensor_copy(ones_bf, scratch_f[:, 0:64])
            ones_r = const.tile([1, 128], F32R, tag="ones_r")
            nc.vector.tensor_copy(ones_r, scratch_f[0:1, :])
            sel2_f = const.tile([2, 128], F32, tag="sel2f")
            nc.sync.dma_start(out=sel2_f, in_=sel2_d.ap())
            sel2 = const.tile([2, 128], F32R, tag="sel2")
            nc.vector.tensor_copy(sel2, sel2_f)
            sel2T_f = const.tile([128, 2], F32, tag="sel2Tf")
            nc.sync.dma_start(out=sel2T_f, in_=sel2T_d.ap())
            sel2T = const.tile([128, 2], F32R, tag="sel2T")
            nc.vector.tensor_copy(sel2T, sel2T_f)

            # exp bias columns (-s_h broadcast to 128 partitions) per head
            nbias_cols = []
            for h in range(cfg.NH):
                col = const.tile([128, 1], F32, tag=f"nb{h}", name=f"nb{h}")
                nc.sync.dma_start(
                    out=col, in_=scl_d.ap()[h, 0:1, :].to_broadcast((128, 1)))
                nbias_cols.append(col)
            # s^2 columns [2,1] per pair (for the k-side scale fold)
            s2_cols = []
            for p in range(PAIRS):
                col = const.tile([2, 1], F32, tag=f"s2_{p}", name=f"s2_{p}")
                nc.sync.dma_start(out=col,
                                  in_=scl_d.ap()[2 * p:2 * p + 2, 1, :])
                s2_cols.append(col)

            bqk_cols = []
            for it in range(2 * PAIRS):
                col = const.tile([128, 1], F32, tag=f"bqk{it}", name=f"bqk{it}")
                nc.sync.dma_start(out=col, in_=bqk_d.ap()[it])
                bqk_cols.append(col)
            bv_r = const.tile([1, VW], F32R, tag="bv")
            nc.sync.dma_start(out=bv_r, in_=bv_d.ap())

            # ---- resident weights ----
            wv_res = const.tile([128, CT, VW], F32R, tag="wv_res")
            nc.sync.dma_start(
                out=wv_res,
                in_=wvT_d.ap().rearrange("(ct p) v -> p ct v", p=128))
            wo_res = const.tile([128, PAIRS, C], BF16, tag="wo_res")
            nc.sync.dma_start(
                out=wo_res,
                in_=woT_d.ap().rearrange("(pt p) c -> p pt c", p=128))

            # ---- xT resident ----
            xt = []
            for ct in range(CT):
                t = xt_pool.tile([128, T], F32R, tag=f"xt{ct}", name=f"xt{ct}")
                nc.sync.dma_start(out=t,
                                  in_=xT_d.ap()[ct * 128:(ct + 1) * 128, :])
                xt.append(t)

            # ================= V projection (f32r, evac casts to bf16) =====
            for tt in range(KT):
                vps = ps_mm.tile([128, VW], F32, tag="mm")
                for ct in range(CT):
                    nc.tensor.matmul(vps, xt[ct][:, tt * 128:(tt + 1) * 128],
                                     wv_res[:, ct, :], start=(ct == 0),
                                     stop=False)
                nc.tensor.matmul(vps, ones_r[:], bv_r[:],
                                 start=False, stop=True)
                vtmp = vtmp_pool.tile([128, VW], BF16, tag="vtmp")
                nc.scalar.copy(vtmp, vps)
                nc.sync.dma_start(out=vhat_sp[tt], in_=vtmp)

            # ============ QK projection + l2norm + scale fold ============
            for p in range(PAIRS):
                for is_k in (0, 1):
                    it = 2 * p + is_k
                    wcol = wcol_pool.tile([128, CT, 128], F32R, tag="wcol")
                    nc.sync.dma_start(
                        out=wcol,
                        in_=wqkT_d.ap().rearrange(
                            "(ct pp) i -> pp ct i", pp=128)[
                                :, :, it * 128:(it + 1) * 128])
                    for tb in range(NQB):
                        ts = slice(tb * QB, (tb + 1) * QB)
                        qs = ps_mm.tile([128, QB], F32, tag="mm")
                        for ct in range(CT):
                            nc.tensor.matmul(qs, wcol[:, ct, :], xt[ct][:, ts],
                                             start=(ct == 0),
                                             stop=(ct == CT - 1))
                        qraw = qk_sb.tile([128, QB], F32, tag="qraw")
                        nc.vector.tensor_scalar_add(qraw, qs, bqk_cols[it])
                        q2 = qk_sb.tile([128, QB], F32R, tag="q2")
                        nc.scalar.activation(q2, qraw, AF.Square)
                        ss = ps_mm.tile([2, QB], F32, tag="mm")
                        nc.tensor.matmul(ss, sel2T[:], q2[:], start=True,
                                         stop=True)
                        ssr = norm_sb.tile([2, QB], F32, tag="ssr")
                        nc.vector.reciprocal_approx_fast(out=ssr, in_=ss)
                        if is_k:
                            nc.vector.tensor_scalar_mul(ssr, ssr, s2_cols[p])
                        rq = norm_sb.tile([2, QB], F32R, tag="rq")
                        nc.scalar.activation(rq, ssr, AF.Sqrt)
                        bc = ps_mm.tile([128, QB], F32, tag="mm")
                        nc.tensor.matmul(bc, sel2[:], rq[:], start=True,
                                         stop=True)
                        qhat = qk_sb.tile([128, QB], F32R, tag="qhat")
                        nc.vector.tensor_mul(qhat, qraw, bc)
                        dst = khat_sp[p] if is_k else qhat_sp[p]
                        nc.sync.dma_start(out=dst[:, ts], in_=qhat)

            # ================= attention =================
            o_fin = {}
            NSG = KT // SG  # exp groups per (pair, qb)
            for p in range(PAIRS):
                kk = att_sb.tile([128, T], F32R, tag="kk")
                nc.sync.dma_start(out=kk, in_=khat_sp[p])
                qq = att_sb.tile([128, T], F32R, tag="qq")
                nc.sync.dma_start(out=qq, in_=qhat_sp[p])
                vv = att_sb.tile([128, KT, 128], BF16, tag="vv")
                nc.sync.dma_start(
                    out=vv,
                    in_=vhat_sp[:, :, p * 128:(p + 1) * 128].rearrange(
                        "kt pp w -> pp kt w"))
                for qb in range(NQB):
                    qsl = slice(qb * QB, (qb + 1) * QB)
                    pv = ps_pv.tile([128, QB], F32, tag="pv")
                    lb = ps_pv.tile([128, QB], F32, tag="lb")
                    for g in range(NSG):
                        sg = ps_sg.tile([128, 2, SG, QB], F32, tag="sg")
                        for j in range(SG):
                            kt = g * SG + j
                            ksl = slice(kt * 128, (kt + 1) * 128)
                            nc.tensor.matmul(sg[:, 0, j, :], kk[0:64, ksl],
                                             qq[0:64, qsl], start=True,
                                             stop=True)
                            nc.tensor.matmul(sg[:, 1, j, :], kk[64:128, ksl],
                                             qq[64:128, qsl], start=True,
                                             stop=True)
                        ptile = pt_pool.tile([128, 2, SG, QB], BF16, tag="pt")
                        if cfg.merge_pairs[p]:
                            nc.scalar.activation(ptile, sg, AF.Exp,
                                                 bias=nbias_cols[2 * p][:])
                        else:
                            nc.scalar.activation(ptile[:, 0], sg[:, 0],
                                                 AF.Exp,
                                                 bias=nbias_cols[2 * p][:])
                            nc.scalar.activation(ptile[:, 1], sg[:, 1],
                                                 AF.Exp,
                                                 bias=nbias_cols[2 * p + 1][:])
                        for j in range(SG):
                            kt = g * SG + j
                            first = kt == 0
                            last = kt == KT - 1
                            nc.tensor.matmul(pv[0:64, :], vv[:, kt, 0:64],
                                             ptile[:, 0, j, :], start=first,
                                             stop=last, tile_position=(0, 0))
                            nc.tensor.matmul(pv[64:128, :], vv[:, kt, 64:128],
                                             ptile[:, 1, j, :], start=first,
                                             stop=last, tile_position=(0, 64),
                                             skip_group_check=True)
                            nc.tensor.matmul(lb[0:64, :], ones_bf[:],
                                             ptile[:, 0, j, :], start=first,
                                             stop=last, tile_position=(0, 0))
                            nc.tensor.matmul(lb[64:128, :], ones_bf[:],
                                             ptile[:, 1, j, :], start=first,
                                             stop=last, tile_position=(0, 64),
                                             skip_group_check=True)
                    rl = att_sb.tile([128, QB], F32, tag="rl")
                    nc.vector.reciprocal_approx_fast(out=rl, in_=lb)
                    of = ofin_pool.tile([128, QB], BF16, tag=f"of{p}_{qb}", name=f"of{p}_{qb}")
                    nc.vector.tensor_mul(of, pv, rl)
                    o_fin[(p, qb)] = of

            # ================= out projection =================
            for qb in range(NQB):
                for cb in range(CT):
                    csl = slice(cb * 128, (cb + 1) * 128)
                    op = ps_mm.tile([128, QB], F32, tag="mm")
                    for p in range(PAIRS):
                        nc.tensor.matmul(op, wo_res[:, p, csl],
                                         o_fin[(p, qb)][:],
                                         start=(p == 0), stop=(p == PAIRS - 1))
                    ot = ot_pool.tile([128, QB], F32, tag="ot")
                    nc.scalar.copy(ot, op)
                    nc.sync.dma_start(
                        out=out_d.ap()[csl, qb * QB:(qb + 1) * QB], in_=ot)

    nc.compile()
    return nc


# ======================= host-side sharding =======================

def shard_inputs(x, w_qkv, b_qkv, w_out, logit_scale):
    """Build per-core input maps. Returns (in_maps, merge_pairs)."""
    x = np.ascontiguousarray(np.asarray(x, dtype=np.float32))
    w_qkv = np.asarray(w_qkv, dtype=np.float32)
    b_qkv = np.asarray(b_qkv, dtype=np.float32)
    w_out = np.asarray(w_out, dtype=np.float32)
    ls = np.asarray(logit_scale, dtype=np.float32).reshape(-1)
    s_all = np.exp(np.minimum(ls, LOG100)).astype(np.float32)  # [HEADS]

    Wq = w_qkv[0 * INNER:1 * INNER]   # [1024, C], row = gh*64+d
    Wk = w_qkv[1 * INNER:2 * INNER]
    Wv = w_qkv[2 * INNER:3 * INNER]
    bq = b_qkv[0 * INNER:1 * INNER]
    bk = b_qkv[1 * INNER:2 * INNER]
    bv = b_qkv[2 * INNER:3 * INNER]

    xT = [np.ascontiguousarray(x[b].T) for b in range(B)]

    per_hg = {}
    merge = [True] * 4
    for hg in range(2):
        heads = list(range(hg * 8, hg * 8 + 8))
        rows, brows = [], []
        for p in range(4):
            g0, g1 = heads[2 * p], heads[2 * p + 1]
            if s_all[g0] != s_all[g1]:
                merge[p] = False
            rows += [Wq[g0 * 64:(g0 + 1) * 64], Wq[g1 * 64:(g1 + 1) * 64],
                     Wk[g0 * 64:(g0 + 1) * 64], Wk[g1 * 64:(g1 + 1) * 64]]
            brows += [bq[g0 * 64:(g0 + 1) * 64], bq[g1 * 64:(g1 + 1) * 64],
                      bk[g0 * 64:(g0 + 1) * 64], bk[g1 * 64:(g1 + 1) * 64]]
        wqkT = np.ascontiguousarray(np.concatenate(rows, axis=0).T)
        bqk = np.ascontiguousarray(
            np.concatenate(brows, axis=0)).reshape(8, 128, 1)
        vsl = slice(hg * 512, (hg + 1) * 512)
        wvT = np.ascontiguousarray(Wv[vsl].T)
        bvs = np.ascontiguousarray(bv[vsl].reshape(1, 512))
        woT = np.ascontiguousarray(w_out[:, vsl].T.astype(ml_bf16))
        scl = np.stack([-s_all[heads], s_all[heads] ** 2],
                       axis=1).astype(np.float32).reshape(8, 2, 1)
        per_hg[hg] = dict(wqkT=wqkT, bqk=bqk, wvT=wvT, bv=bvs, woT=woT,
                          scl=scl)

    sel2 = np.zeros((2, 128), dtype=np.float32)
    sel2[0, 0:64] = 1.0
    sel2[1, 64:128] = 1.0
    sel2T = np.ascontiguousarray(sel2.T)
    in_maps = []
    for c in range(N_CORES):
        b, hg = c // 2, c % 2
        m = dict(per_hg[hg])
        m["xT"] = xT[b]
        m["sel2"] = sel2
        m["sel2T"] = sel2T
        in_maps.append(m)
    return in_maps, tuple(merge)


_NC_CACHE = {}
TRACE = False
LAST_RESULT = None


def kernel(x, w_qkv, b_qkv, w_out, logit_scale):
    global LAST_RESULT
    in_maps, merge_pairs = shard_inputs(x, w_qkv, b_qkv, w_out, logit_scale)
    cfg = Cfg(merge_pairs=merge_pairs)
    if merge_pairs not in _NC_CACHE:
        _NC_CACHE[merge_pairs] = build(cfg)
    nc = _NC_CACHE[merge_pairs]
    res = run_bass_kernel_spmd(nc, in_maps, core_ids=list(range(N_CORES)),
                               trace=TRACE)
    LAST_RESULT = res
    outs = [res.results[c]["out"] for c in range(N_CORES)]
    full = np.empty((B, N, D_MODEL), dtype=np.float32)
    for b in range(B):
        full[b] = (outs[2 * b] + outs[2 * b + 1]).T
    return full
